# revision 25
# baseline (speedup 1.0000x reference)
"""Trainium2 Bass kernel for nn_Block_71528385347823 (dense transformer block).

Sharding (8 NeuronCores, 1 chip):
  - LN1 row-parallel (512 rows/core) -> AllGather of h^T (bf16)
  - Attention head-parallel: core c owns heads {2c, 2c+1}, all tokens
  - Per-batch AllToAll of per-head attention outputs y^T
  - out-proj / residual / LN2 / MLP row-parallel (256 rows per batch per core)
Pipelined per batch: attention(b1) overlaps MLP(b0).
All matmuls bf16 with fp32 PSUM accumulation.
"""

import os
import sys

if "/opt/trn_rl_repo" not in sys.path:
    sys.path.insert(0, "/opt/trn_rl_repo")

import numpy as np
import ml_dtypes

import concourse.bass as bass
import concourse.bacc as bacc_mod
import concourse.mybir as mybir
import concourse.tile as tile
from concourse.bass_utils import run_bass_kernel_spmd
from concourse.masks import make_identity

F32 = mybir.dt.float32
BF16 = mybir.dt.bfloat16
AF = mybir.ActivationFunctionType
ALU = mybir.AluOpType

N_CORES = 8
B, T, C, H, D = 2, 2048, 1024, 16, 64
HPC = H // N_CORES          # heads per core = 2
ROWS = (B * T) // N_CORES   # rows per core = 512 (256 per batch)
RPB = ROWS // B             # rows per batch per core = 256
NT = B * T
KT = C // 128               # 8 contraction k-tiles over C
SCALE = C ** -0.5
EPS = 1e-5
TQ = 512                    # query-chunk width
TB = 128                    # ts block size
NBLK = T // TB              # 16 t-blocks per batch
NCH = T // TQ               # 4 q-chunks per batch

bf16 = ml_dtypes.bfloat16

last_results = None


def _ln_apply(nc, pool, x_tile, h_out, eps_tile):
    """LayerNorm (no affine) on a [128, 1024] fp32 tile -> h_out (bf16)."""
    stats = pool.tile([128, 2, 6], F32, tag="ln_stats", name="ln_stats")
    nc.vector.bn_stats(stats[:, 0, :], x_tile[:, 0:512])
    nc.vector.bn_stats(stats[:, 1, :], x_tile[:, 512:1024])
    mv = pool.tile([128, 2], F32, tag="ln_mv", name="ln_mv")
    nc.vector.bn_aggr(mv, stats)
    rstd = pool.tile([128, 1], F32, tag="ln_rstd", name="ln_rstd")
    nc.scalar.activation(rstd, mv[:, 1:2], AF.Sqrt, bias=eps_tile, scale=1.0)
    nc.vector.reciprocal(rstd, rstd)
    nc.vector.tensor_scalar(
        out=h_out, in0=x_tile, scalar1=mv[:, 0:1], scalar2=rstd,
        op0=ALU.subtract, op1=ALU.mult,
    )


def build_program(add_bo: bool, add_bb2: bool) -> bass.Bass:
    nc = bacc_mod.Bacc(None, num_devices=N_CORES, target_bir_lowering=False)
    RG = [list(range(N_CORES))]

    # ---------------- I/O ----------------
    # x_loc rows: [batch0 rows 256c..256(c+1)) | batch1 rows 256c..256(c+1))]
    x_loc = nc.dram_tensor("x_loc", [ROWS, C], F32, kind="ExternalInput")
    wq = nc.dram_tensor("wq", [C, 128], BF16, kind="ExternalInput")
    wk = nc.dram_tensor("wk", [C, 128], BF16, kind="ExternalInput")
    wv = nc.dram_tensor("wv", [C, 128], BF16, kind="ExternalInput")
    qb = nc.dram_tensor("qb", [128, 1], F32, kind="ExternalInput")
    kb = nc.dram_tensor("kb", [128, 1], F32, kind="ExternalInput")
    wo = nc.dram_tensor("wo", [C, C], BF16, kind="ExternalInput")
    w1 = nc.dram_tensor("w1", [C, 4 * C], BF16, kind="ExternalInput")
    gelub = nc.dram_tensor("gelub", [128, 32], F32, kind="ExternalInput")
    w2 = nc.dram_tensor("w2", [4 * C, C], BF16, kind="ExternalInput")
    maskin = nc.dram_tensor("maskin", [128, 128], BF16, kind="ExternalInput")
    if add_bo:
        boeff = nc.dram_tensor("boeff", [1, C], F32, kind="ExternalInput")
    if add_bb2:
        bb2v = nc.dram_tensor("bb2v", [1, C], F32, kind="ExternalInput")
    out_loc = nc.dram_tensor("out_loc", [ROWS, C], F32, kind="ExternalOutput")

    # collective bounce buffers (internal DRAM)
    hT_bounce = nc.dram_tensor("hT_bounce", [C, ROWS], BF16)
    hT_all = [nc.dram_tensor(f"hT_all{i}", [N_CORES, C // 2, ROWS], BF16,
                             addr_space="Shared") for i in range(2)]
    y_bounce = [nc.dram_tensor(f"y_bounce{b}", [N_CORES, 128, RPB], BF16)
                for b in range(B)]
    y_all = [nc.dram_tensor(f"y_all{b}", [N_CORES, 128, RPB], BF16)
             for b in range(B)]
    recip_dram = [nc.dram_tensor(f"recip_dram{b}", [64, 64], BF16)
                  for b in range(B)]

    with tile.TileContext(nc) as tc, \
         tc.tile_pool(name="const", bufs=1) as const_p, \
         tc.tile_pool(name="small", bufs=4) as small, \
         tc.tile_pool(name="psT", bufs=1, space="PSUM") as psT, \
         tc.tile_pool(name="mmps", bufs=5, space="PSUM") as mmps:

        identity = const_p.tile([128, 128], BF16)
        make_identity(nc, identity)
        eps_tile = const_p.tile([128, 1], F32)
        nc.vector.memset(eps_tile, EPS)
        mask_sb = const_p.tile([128, 128], BF16)
        nc.scalar.dma_start(out=mask_sb, in_=maskin[:, :])
        qb_sb = const_p.tile([128, 1], F32)
        nc.scalar.dma_start(out=qb_sb, in_=qb[:, :])
        kb_sb = const_p.tile([128, 1], F32)
        nc.scalar.dma_start(out=kb_sb, in_=kb[:, :])
        gelub_sb = const_p.tile([128, 32], F32)
        nc.scalar.dma_start(out=gelub_sb, in_=gelub[:, :])
        if add_bo:
            bo_rep = const_p.tile([128, C], F32)
            nc.sync.dma_start(out=bo_rep, in_=boeff[0:1, :].to_broadcast([128, C]))
        if add_bb2:
            bb2_rep = const_p.tile([128, C], F32)
            nc.sync.dma_start(out=bb2_rep, in_=bb2v[0:1, :].to_broadcast([128, C]))

        # activations (x streamed on demand from HBM, never resident)
        xp = tc.alloc_tile_pool(name="xp", bufs=1)
        x2_sb = [xp.tile([128, C], F32, tag=f"x2_{m}", name=f"x2_{m}")
                 for m in range(4)]
        h2T_sb = xp.tile([128, KT, ROWS], BF16, tag="h2T", name="h2T")

        # out-proj weight (2nd DMA ring; needed from mid-kernel)
        wop = tc.alloc_tile_pool(name="wop", bufs=1)
        wo_sb = wop.tile([128, KT, C], BF16, name="wo_sb")
        nc.scalar.dma_start(out=wo_sb, in_=wo[:, :].rearrange("(k p) n -> p k n", p=128))

        # ======== shared attention state =========
        yiop = tc.alloc_tile_pool(name="yio", bufs=1)
        ynorm = [yiop.tile([128, T], BF16, tag=f"yn{b}", name=f"yn{b}")
                 for b in range(B)]
        sums_pack = [yiop.tile([128, 64], F32, tag=f"sp{b}", name=f"sp{b}")
                     for b in range(B)]
        yT_recv = [yiop.tile([128, N_CORES, RPB], BF16, tag=f"yr{b}",
                             name=f"yr{b}") for b in range(B)]

        psY = tc.alloc_tile_pool(name="psY", bufs=1, space="PSUM")
        p5p = tc.alloc_tile_pool(name="p5p", bufs=2)
        attn_pools = [tc.alloc_tile_pool(name=f"attn{b}", bufs=1)
                      for b in range(B)]
        xr_p = tc.alloc_tile_pool(name="xr", bufs=3)
        # qkv per-head-pair weights (freed after QKV)
        wqkv_p = tc.alloc_tile_pool(name="wqkv", bufs=1)
        wq_sb = wqkv_p.tile([128, KT, 128], BF16, name="wq_sb")
        nc.scalar.dma_start(out=wq_sb, in_=wq[:, :].rearrange("(k p) m -> p k m", p=128))
        wk_sb = wqkv_p.tile([128, KT, 128], BF16, name="wk_sb")
        nc.scalar.dma_start(out=wk_sb, in_=wk[:, :].rearrange("(k p) m -> p k m", p=128))
        wv_sb = wqkv_p.tile([128, KT, 128], BF16, name="wv_sb")
        nc.scalar.dma_start(out=wv_sb, in_=wv[:, :].rearrange("(k p) m -> p k m", p=128))
        hTe_p = tc.alloc_tile_pool(name="hTe", bufs=2)


        # =======================================================
        # P2: LN1 on local rows, PE-transpose to h^T, AllGather
        # =======================================================
        with tc.tile_pool(name="p2", bufs=4) as p2, \
             tc.tile_pool(name="p2big", bufs=1) as p2big:
            hT_sb = p2big.tile([128, KT, ROWS], BF16)
            for m in range(4):
                xt = xr_p.tile([128, C], F32, tag="xr", name="xr")
                nc.sync.dma_start(out=xt, in_=x_loc[128 * m:128 * (m + 1), :])
                h_t = p2.tile([128, C], BF16, tag="h", name="h_t")
                _ln_apply(nc, p2, xt, h_t, eps_tile)
                for kp in range(2):  # two groups of 4 k-tiles
                    pst = psT.tile([128, 4, 128], BF16, tag="tps", name="pst")
                    for kk in range(4):
                        k = 4 * kp + kk
                        nc.tensor.transpose(
                            pst[:, kk, :], h_t[:, 128 * k:128 * (k + 1)], identity)
                    nc.vector.tensor_copy(
                        out=hT_sb[:, 4 * kp:4 * (kp + 1), 128 * m:128 * (m + 1)],
                        in_=pst)
            for half in range(2):
                for kk in range(4):
                    k = 4 * half + kk
                    nc.sync.dma_start(
                        out=hT_bounce[128 * k:128 * (k + 1), :],
                        in_=hT_sb[:, k, :])
                nc.gpsimd.collective_compute(
                    "AllGather", ALU.bypass, replica_groups=RG,
                    ins=[hT_bounce[512 * half:512 * (half + 1), :]],
                    outs=[hT_all[half][:, :, :]],
                )

        # w1/w2 MLP weight streaming pools (allocated late, in pipeline)
        mlp_pools = {}

        def qkv_batch(b):
            """QKV projections for batch b (4 chunks of 512 tokens)."""
            ap = attn_pools[b]
            qT = ap.tile([128, T], BF16, tag="qT", name=f"qTb{b}")
            kT = ap.tile([128, T], BF16, tag="kT", name=f"kTb{b}")
            vv = ap.tile([128, NBLK, 130], BF16, tag="vv", name=f"vvb{b}")
            nc.gpsimd.memset(vv[:, :, 64:65], 1.0)
            nc.gpsimd.memset(vv[:, :, 129:130], 1.0)
            for le in range(NCH):
                hTe = hTe_p.tile([128, KT, TQ], BF16, tag="hTe", name="hTe")
                # batch-b tokens [512*le, 512*(le+1)) live as rows
                # [256*b, 256*(b+1)) of cores 2*le and 2*le+1
                for half in range(2):
                    for hh in range(2):
                        nc.sync.dma_start(
                            out=hTe[:, 4 * half:4 * (half + 1),
                                    RPB * hh:RPB * (hh + 1)],
                            in_=hT_all[half][2 * le + hh][:, RPB * b:RPB * (b + 1)]
                            .rearrange("(k p) t -> p k t", p=128))
                for which, w_sb, bias_sb, dst in (
                    ("q", wq_sb, qb_sb, qT), ("k", wk_sb, kb_sb, kT),
                ):
                    ps = mmps.tile([128, TQ], F32, tag="mm", name=f"ps{which}")
                    for k in range(KT):
                        nc.tensor.matmul(ps, lhsT=w_sb[:, k, :], rhs=hTe[:, k, :],
                                         start=(k == 0), stop=(k == KT - 1))
                    nc.scalar.activation(
                        out=dst[:, TQ * le:TQ * (le + 1)], in_=ps,
                        func=AF.Identity, bias=bias_sb, scale=1.0)
                psv = mmps.tile([128, TQ], F32, tag="mm", name="psv")
                for k in range(KT):
                    nc.tensor.matmul(psv, lhsT=wv_sb[:, k, :], rhs=hTe[:, k, :],
                                     start=(k == 0), stop=(k == KT - 1))
                vTc = hTe_p.tile([128, TQ], BF16, tag="vTc", name="vTc")
                nc.scalar.copy(out=vTc, in_=psv)
                pst = psT.tile([128, 4, 128], BF16, tag="tps", name="pstv")
                for sb in range(4):
                    nc.tensor.transpose(
                        pst[:, sb, :], vTc[:, 128 * sb:128 * (sb + 1)], identity)
                tb0 = le * 4
                nc.vector.tensor_copy(
                    out=vv[:, tb0:tb0 + 4, 0:64], in_=pst[:, :, 0:64])
                nc.vector.tensor_copy(
                    out=vv[:, tb0:tb0 + 4, 65:129], in_=pst[:, :, 64:128])
            return qT, kT, vv

        def attn_batch(b, qT, kT, vv):
            """Causal attention for batch b; writes ynorm[b] (unnormalized
            for now) and sums_pack[b]."""
            for j in range(NCH):
                tq0 = TQ * j
                psy = [psY.tile([128, TQ], F32, tag=f"y{h}", name=f"psy{h}")
                       for h in range(HPC)]
                nblocks = 4 * j + 4
                for i in range(nblocks):
                    qoff = max(0, TB * (i - 4 * j))
                    for h in range(HPC):
                        pss = mmps.tile([128, TQ], F32, tag="mm", name=f"pss{h}")
                        nc.tensor.matmul(
                            pss[:, qoff:TQ],
                            lhsT=kT[64 * h:64 * (h + 1), TB * i:TB * (i + 1)],
                            rhs=qT[64 * h:64 * (h + 1), tq0 + qoff:tq0 + TQ],
                            start=True, stop=True)
                        pt = p5p.tile([128, TQ], BF16, tag=f"p{h}", name=f"pt{h}", bufs=3)
                        nc.scalar.activation(
                            out=pt[:, qoff:TQ], in_=pss[:, qoff:TQ],
                            func=AF.Exp, bias=0.0, scale=SCALE)
                        if i >= 4 * j:
                            nc.vector.tensor_mul(
                                out=pt[:, qoff:qoff + TB],
                                in0=pt[:, qoff:qoff + TB], in1=mask_sb)
                        nc.tensor.matmul(
                            psy[h][0:65, qoff:TQ],
                            lhsT=vv[:, i, 65 * h:65 * (h + 1)],
                            rhs=pt[:, qoff:TQ],
                            start=(i == 0), stop=(i == nblocks - 1))
                for h in range(HPC):
                    cb = h * NCH + j  # 0..7 within batch
                    srow = small.tile([1, TQ], F32, tag="srow", name="srow")
                    nc.vector.tensor_copy(out=srow, in_=psy[h][64:65, 0:TQ])
                    nc.sync.dma_start(
                        out=sums_pack[b][8 * cb:8 * (cb + 1), 0:64], in_=srow)
                    nc.vector.tensor_copy(
                        out=ynorm[b][64 * h:64 * (h + 1), tq0:tq0 + TQ],
                        in_=psy[h][0:64, :])

        def normalize_a2a(b):
            """Reciprocal + normalize ynorm[b] in place, bounce + AllToAll."""
            recip = small.tile([128, 64], F32, tag="recip", name="recip")
            nc.vector.reciprocal(recip[0:64, :], sums_pack[b][0:64, :])
            recip_bf = small.tile([128, 64], BF16, tag="recipb", name="recipb")
            nc.vector.tensor_copy(recip_bf[0:64, :], recip[0:64, :])
            nc.sync.dma_start(out=recip_dram[b][:, :], in_=recip_bf[0:64, :])
            for j in range(NCH):
                tq0 = TQ * j
                rep = p5p.tile([128, TQ], BF16, tag="rep", name="rep")
                for h in range(HPC):
                    cb = h * NCH + j
                    bc = bass.AP(tensor=recip_dram[b], offset=TQ * cb,
                                 ap=[[0, 64], [1, TQ]])
                    nc.sync.dma_start(out=rep[64 * h:64 * (h + 1), :], in_=bc)
                nc.vector.tensor_mul(
                    out=ynorm[b][:, tq0:tq0 + TQ],
                    in0=ynorm[b][:, tq0:tq0 + TQ], in1=rep)
            for e in range(N_CORES):
                nc.sync.dma_start(
                    out=y_bounce[b][e], in_=ynorm[b][:, RPB * e:RPB * (e + 1)])
            nc.gpsimd.collective_compute(
                "AllToAll", ALU.bypass, replica_groups=RG,
                ins=[y_bounce[b][:, :, :]], outs=[y_all[b][:, :, :]],
            )
            nc.sync.dma_start(
                out=yT_recv[b], in_=y_all[b][:, :, :].rearrange("e p t -> p e t"))

        def mlp_front(b):
            """out-proj + residual + LN2 + mm1 + gelu for batch b rows."""
            for m in range(2):
                xm = 2 * b + m
                xt = xr_p.tile([128, C], F32, tag="xr", name="xr2")
                nc.sync.dma_start(
                    out=xt, in_=x_loc[128 * xm:128 * (xm + 1), :])
                for n in range(2):
                    ps = mmps.tile([128, TQ], F32, tag="mm", name="pso")
                    for e in range(N_CORES):
                        nc.tensor.matmul(
                            ps, lhsT=yT_recv[b][:, e, 128 * m:128 * (m + 1)],
                            rhs=wo_sb[:, e, TQ * n:TQ * (n + 1)],
                            start=(e == 0), stop=(e == N_CORES - 1))
                    nc.vector.tensor_add(
                        out=x2_sb[xm][:, TQ * n:TQ * (n + 1)],
                        in0=xt[:, TQ * n:TQ * (n + 1)], in1=ps)
                if add_bo:
                    nc.vector.tensor_add(
                        out=x2_sb[xm], in0=x2_sb[xm], in1=bo_rep)
                h2 = p5p.tile([128, C], BF16, tag="h2", name="h2")
                _ln_apply(nc, small, x2_sb[xm], h2, eps_tile)
                for kp in range(2):
                    pst = psT.tile([128, 4, 128], BF16, tag="tps", name="psth")
                    for kk in range(4):
                        k = 4 * kp + kk
                        nc.tensor.transpose(
                            pst[:, kk, :], h2[:, 128 * k:128 * (k + 1)], identity)
                    nc.vector.tensor_copy(
                        out=h2T_sb[:, 4 * kp:4 * (kp + 1), 128 * xm:128 * (xm + 1)],
                        in_=pst)
            # mm1 + gelu over this batch's 256 rows, W1 streamed in quarters
            w1p, gTp, mlp_p = (mlp_pools[k] for k in ("w1p", "gTp", "mlp"))
            gT = gTp.tile([128, 32, RPB], BF16, tag="gT", name=f"gT{b}")
            r0 = RPB * b
            for uq in range(8):
                w1q = w1p.tile([128, KT, 512], BF16, tag="w1q", name="w1q")
                nc.scalar.dma_start(
                    out=w1q,
                    in_=w1[:, 512 * uq:512 * (uq + 1)]
                    .rearrange("(k p) n -> p k n", p=128))
                for u8 in range(4):
                    u = 4 * uq + u8
                    ps = mmps.tile([128, RPB], F32, tag="mm", name="psu")
                    for k in range(KT):
                        nc.tensor.matmul(
                            ps, lhsT=w1q[:, k, 128 * u8:128 * (u8 + 1)],
                            rhs=h2T_sb[:, k, r0:r0 + RPB],
                            start=(k == 0), stop=(k == KT - 1))
                    nc.scalar.activation(
                        out=gT[:, u, :], in_=ps, func=AF.Gelu_apprx_tanh,
                        bias=gelub_sb[:, u:u + 1], scale=1.0)
            return gT

        def mlp_mm2(b, gT):
            # mm2: half-outer with streamed W2 half, partial accum in out tile
            mlp_p = mlp_pools["mlp"]
            ot_t = [mlp_p.tile([128, C], F32, tag=f"ot{m}", name=f"ot{m}")
                    for m in range(2)]
            for half in range(2):
                w2h = w2p.tile([128, 16, C], BF16, tag="w2h", name="w2h")
                nc.scalar.dma_start(
                    out=w2h, in_=w2[2048 * half:2048 * (half + 1), :]
                    .rearrange("(k p) n -> p k n", p=128))
                for m in range(2):
                    xm = 2 * b + m
                    for n in range(2):
                        ps = mmps.tile([128, TQ], F32, tag="mm", name="psm")
                        for kk in range(16):
                            u = 16 * half + kk
                            nc.tensor.matmul(
                                ps, lhsT=gT[:, u, 128 * m:128 * (m + 1)],
                                rhs=w2h[:, kk, TQ * n:TQ * (n + 1)],
                                start=(kk == 0), stop=(kk == 15))
                        if half == 0:
                            nc.vector.tensor_add(
                                out=ot_t[m][:, TQ * n:TQ * (n + 1)],
                                in0=x2_sb[xm][:, TQ * n:TQ * (n + 1)], in1=ps)
                        else:
                            nc.vector.tensor_add(
                                out=ot_t[m][:, TQ * n:TQ * (n + 1)],
                                in0=ot_t[m][:, TQ * n:TQ * (n + 1)], in1=ps)
            for m in range(2):
                xm = 2 * b + m
                if add_bb2:
                    nc.vector.tensor_add(out=ot_t[m], in0=ot_t[m], in1=bb2_rep)
                nc.sync.dma_start(
                    out=out_loc[128 * xm:128 * (xm + 1), :], in_=ot_t[m])

        # ---------------- pipeline ----------------
        qT0, kT0, vv0 = qkv_batch(0)
        attn_batch(0, qT0, kT0, vv0)
        qT1, kT1, vv1 = qkv_batch(1)       # fills PE idle during attn(0)
        hTe_p.release()
        wqkv_p.release()
        mlp_pools["w1p"] = tc.alloc_tile_pool(name="w1p", bufs=2)
        mlp_pools["gTp"] = tc.alloc_tile_pool(name="gTp", bufs=1)
        mlp_pools["mlp"] = tc.alloc_tile_pool(name="mlp", bufs=2)
        w2p = tc.alloc_tile_pool(name="w2p", bufs=1)
        normalize_a2a(0)
        attn_batch(1, qT1, kT1, vv1)
        gT0 = mlp_front(0)                  # overlaps attn(1)
        normalize_a2a(1)
        mlp_mm2(0, gT0)                     # covers A2A(1) transit
        gT1 = mlp_front(1)
        mlp_mm2(1, gT1)
        for pool in (w2p, mlp_pools["mlp"], mlp_pools["gTp"],
                     mlp_pools["w1p"], xr_p, attn_pools[1], attn_pools[0],
                     p5p, psY, yiop, wop, xp):
            pool.release()

    nc.finalize()
    return nc


_program_cache = {}


def kernel(**inputs) -> np.ndarray:
    global last_results
    x = np.asarray(inputs["x"], np.float32)
    Wq = np.asarray(inputs["Wq"], np.float32)
    Wk = np.asarray(inputs["Wk"], np.float32)
    Wv = np.asarray(inputs["Wv"], np.float32)
    Wo = np.asarray(inputs["Wo"], np.float32)
    bo = np.asarray(inputs["bo"], np.float32)
    g1 = np.asarray(inputs["g1"], np.float32)
    b1 = np.asarray(inputs["b1"], np.float32)
    g2 = np.asarray(inputs["g2"], np.float32)
    b2 = np.asarray(inputs["b2"], np.float32)
    W1 = np.asarray(inputs["W1"], np.float32)
    bb1 = np.asarray(inputs["bb1"], np.float32)
    W2 = np.asarray(inputs["W2"], np.float32)
    bb2 = np.asarray(inputs["bb2"], np.float32)

    xb = x.reshape(B, T, C)

    vb_full = np.einsum("c,hcd->hd", b1, Wv).reshape(C)
    boeff = bo + vb_full @ Wo
    gelu_bias = b2 @ (g2[:, None] * W1) + bb1
    add_bo = bool(np.abs(boeff).max() > 0)
    add_bb2 = bool(np.abs(bb2).max() > 0)

    key = (add_bo, add_bb2)
    if key not in _program_cache:
        _program_cache[key] = build_program(add_bo, add_bb2)
    nc = _program_cache[key]

    wo_b = Wo.astype(bf16)
    w1_b = (g2[:, None] * W1).astype(bf16)
    w2_b = W2.astype(bf16)
    gelub_np = np.ascontiguousarray(
        gelu_bias.reshape(32, 128).T).astype(np.float32)
    s_idx = np.arange(128)[:, None]
    q_idx = np.arange(128)[None, :]
    mask_np = (s_idx <= q_idx).astype(bf16)

    in_maps = []
    for c in range(N_CORES):
        hsl = slice(HPC * c, HPC * (c + 1))
        Wqh = (g1[:, None] * Wq[hsl]).transpose(1, 0, 2).reshape(C, 128)
        Wkh = (g1[:, None] * Wk[hsl]).transpose(1, 0, 2).reshape(C, 128)
        Wvh = (g1[:, None] * Wv[hsl]).transpose(1, 0, 2).reshape(C, 128)
        qb_np = np.einsum("c,hcd->hd", b1, Wq[hsl]).reshape(128, 1)
        kb_np = np.einsum("c,hcd->hd", b1, Wk[hsl]).reshape(128, 1)
        x_core = np.concatenate(
            [xb[b, RPB * c:RPB * (c + 1), :] for b in range(B)], axis=0)
        m = {
            "x_loc": np.ascontiguousarray(x_core),
            "wq": np.ascontiguousarray(Wqh).astype(bf16),
            "wk": np.ascontiguousarray(Wkh).astype(bf16),
            "wv": np.ascontiguousarray(Wvh).astype(bf16),
            "qb": qb_np.astype(np.float32), "kb": kb_np.astype(np.float32),
            "wo": wo_b, "w1": w1_b, "w2": w2_b,
            "gelub": gelub_np, "maskin": mask_np,
        }
        if add_bo:
            m["boeff"] = boeff.reshape(1, C).astype(np.float32)
        if add_bb2:
            m["bb2v"] = bb2.reshape(1, C).astype(np.float32)
        in_maps.append(m)

    trace = bool(int(os.environ.get("KERNEL_TRACE", "0")))
    res = run_bass_kernel_spmd(nc, in_maps, core_ids=list(range(N_CORES)),
                               trace=trace)
    last_results = res
    out = np.empty((B, T, C), np.float32)
    for c in range(N_CORES):
        r = res.results[c]["out_loc"]
        for b in range(B):
            out[b, RPB * c:RPB * (c + 1), :] = r[RPB * b:RPB * (b + 1), :]
    return out


# revision 27
# speedup vs baseline: 1.0767x; 1.0767x over previous
"""Trainium2 Bass kernel for nn_Block_71528385347823 (dense transformer block).

Sharding (8 NeuronCores, 1 chip):
  - LN1 row-parallel (512 rows/core) -> AllGather of h^T (bf16)
  - Attention head-parallel: core c owns heads {2c, 2c+1}, all tokens
  - Per-batch AllToAll of per-head attention outputs y^T
  - out-proj / residual / LN2 / MLP row-parallel (256 rows per batch per core)
Pipelined per batch: attention(b1) overlaps MLP(b0).
All matmuls bf16 with fp32 PSUM accumulation.
"""

import os
import sys

if "/opt/trn_rl_repo" not in sys.path:
    sys.path.insert(0, "/opt/trn_rl_repo")

import numpy as np
import ml_dtypes

import concourse.bass as bass
import concourse.bacc as bacc_mod
import concourse.mybir as mybir
import concourse.tile as tile
from concourse.bass_utils import run_bass_kernel_spmd
from concourse.masks import make_identity

F32 = mybir.dt.float32
BF16 = mybir.dt.bfloat16
AF = mybir.ActivationFunctionType
ALU = mybir.AluOpType

N_CORES = 8
B, T, C, H, D = 2, 2048, 1024, 16, 64
HPC = H // N_CORES          # heads per core = 2
ROWS = (B * T) // N_CORES   # rows per core = 512 (256 per batch)
RPB = ROWS // B             # rows per batch per core = 256
NT = B * T
KT = C // 128               # 8 contraction k-tiles over C
SCALE = C ** -0.5
EPS = 1e-5
TQ = 512                    # query-chunk width
TB = 128                    # ts block size
NBLK = T // TB              # 16 t-blocks per batch
NCH = T // TQ               # 4 q-chunks per batch

bf16 = ml_dtypes.bfloat16

last_results = None


def _ln_apply(nc, pool, x_tile, h_out, eps_tile):
    """LayerNorm (no affine) on a [128, 1024] fp32 tile -> h_out (bf16)."""
    stats = pool.tile([128, 2, 6], F32, tag="ln_stats", name="ln_stats")
    nc.vector.bn_stats(stats[:, 0, :], x_tile[:, 0:512])
    nc.vector.bn_stats(stats[:, 1, :], x_tile[:, 512:1024])
    mv = pool.tile([128, 2], F32, tag="ln_mv", name="ln_mv")
    nc.vector.bn_aggr(mv, stats)
    rstd = pool.tile([128, 1], F32, tag="ln_rstd", name="ln_rstd")
    nc.scalar.activation(rstd, mv[:, 1:2], AF.Sqrt, bias=eps_tile, scale=1.0)
    nc.vector.reciprocal(rstd, rstd)
    nc.vector.tensor_scalar(
        out=h_out, in0=x_tile, scalar1=mv[:, 0:1], scalar2=rstd,
        op0=ALU.subtract, op1=ALU.mult,
    )


def build_program(add_bo: bool, add_bb2: bool) -> bass.Bass:
    nc = bacc_mod.Bacc(None, num_devices=N_CORES, target_bir_lowering=False)
    RG = [list(range(N_CORES))]

    # ---------------- I/O ----------------
    # x_loc rows: [batch0 rows 256c..256(c+1)) | batch1 rows 256c..256(c+1))]
    x_loc = nc.dram_tensor("x_loc", [ROWS, C], F32, kind="ExternalInput")
    wq = nc.dram_tensor("wq", [C, 128], BF16, kind="ExternalInput")
    wk = nc.dram_tensor("wk", [C, 128], BF16, kind="ExternalInput")
    wv = nc.dram_tensor("wv", [C, 128], BF16, kind="ExternalInput")
    qb = nc.dram_tensor("qb", [128, 1], F32, kind="ExternalInput")
    kb = nc.dram_tensor("kb", [128, 1], F32, kind="ExternalInput")
    wo = nc.dram_tensor("wo", [C, C], BF16, kind="ExternalInput")
    w1 = nc.dram_tensor("w1", [C, 4 * C], BF16, kind="ExternalInput")
    gelub = nc.dram_tensor("gelub", [128, 32], F32, kind="ExternalInput")
    w2 = nc.dram_tensor("w2", [4 * C, C], BF16, kind="ExternalInput")
    maskin = nc.dram_tensor("maskin", [128, 128], BF16, kind="ExternalInput")
    if add_bo:
        boeff = nc.dram_tensor("boeff", [1, C], F32, kind="ExternalInput")
    if add_bb2:
        bb2v = nc.dram_tensor("bb2v", [1, C], F32, kind="ExternalInput")
    out_loc = nc.dram_tensor("out_loc", [ROWS, C], F32, kind="ExternalOutput")

    # collective bounce buffers (internal DRAM)
    hT_bounce = nc.dram_tensor("hT_bounce", [C, ROWS], BF16)
    hT_all = nc.dram_tensor("hT_all", [N_CORES, C, ROWS], BF16,
                            addr_space="Shared")
    y_bounce = [nc.dram_tensor(f"y_bounce{b}", [N_CORES, 128, RPB], BF16)
                for b in range(B)]
    y_all = [nc.dram_tensor(f"y_all{b}", [N_CORES, 128, RPB], BF16)
             for b in range(B)]
    recip_dram = [nc.dram_tensor(f"recip_dram{b}", [64, 64], BF16)
                  for b in range(B)]

    with tile.TileContext(nc) as tc, \
         tc.tile_pool(name="const", bufs=1) as const_p, \
         tc.tile_pool(name="small", bufs=4) as small, \
         tc.tile_pool(name="psT", bufs=1, space="PSUM") as psT, \
         tc.tile_pool(name="mmps", bufs=5, space="PSUM") as mmps:

        identity = const_p.tile([128, 128], BF16)
        make_identity(nc, identity)
        eps_tile = const_p.tile([128, 1], F32)
        nc.vector.memset(eps_tile, EPS)
        mask_sb = const_p.tile([128, 128], BF16)
        nc.scalar.dma_start(out=mask_sb, in_=maskin[:, :])
        qb_sb = const_p.tile([128, 1], F32)
        nc.scalar.dma_start(out=qb_sb, in_=qb[:, :])
        kb_sb = const_p.tile([128, 1], F32)
        nc.scalar.dma_start(out=kb_sb, in_=kb[:, :])
        gelub_sb = const_p.tile([128, 32], F32)
        nc.scalar.dma_start(out=gelub_sb, in_=gelub[:, :])
        if add_bo:
            bo_rep = const_p.tile([128, C], F32)
            nc.sync.dma_start(out=bo_rep, in_=boeff[0:1, :].to_broadcast([128, C]))
        if add_bb2:
            bb2_rep = const_p.tile([128, C], F32)
            nc.sync.dma_start(out=bb2_rep, in_=bb2v[0:1, :].to_broadcast([128, C]))

        # activations (x streamed on demand from HBM, never resident)
        xp = tc.alloc_tile_pool(name="xp", bufs=1)
        x2_sb = [xp.tile([128, C], F32, tag=f"x2_{m}", name=f"x2_{m}")
                 for m in range(4)]
        h2T_sb = xp.tile([128, KT, ROWS], BF16, tag="h2T", name="h2T")

        # out-proj weight (2nd DMA ring; needed from mid-kernel)
        wop = tc.alloc_tile_pool(name="wop", bufs=1)
        wo_sb = wop.tile([128, KT, C], BF16, name="wo_sb")
        nc.scalar.dma_start(out=wo_sb, in_=wo[:, :].rearrange("(k p) n -> p k n", p=128))

        # ======== shared attention state =========
        yiop = tc.alloc_tile_pool(name="yio", bufs=1)
        ynorm = [yiop.tile([128, T], BF16, tag=f"yn{b}", name=f"yn{b}")
                 for b in range(B)]
        sums_pack = [yiop.tile([128, 64], F32, tag=f"sp{b}", name=f"sp{b}")
                     for b in range(B)]
        yT_recv = [yiop.tile([128, N_CORES, RPB], BF16, tag=f"yr{b}",
                             name=f"yr{b}") for b in range(B)]

        psY = tc.alloc_tile_pool(name="psY", bufs=1, space="PSUM")
        p5p = tc.alloc_tile_pool(name="p5p", bufs=2)
        attn_pools = [tc.alloc_tile_pool(name=f"attn{b}", bufs=1)
                      for b in range(B)]
        xr_p = tc.alloc_tile_pool(name="xr", bufs=3)
        # qkv per-head-pair weights (freed after QKV)
        wqkv_p = tc.alloc_tile_pool(name="wqkv", bufs=1)
        wq_sb = wqkv_p.tile([128, KT, 128], BF16, name="wq_sb")
        nc.scalar.dma_start(out=wq_sb, in_=wq[:, :].rearrange("(k p) m -> p k m", p=128))
        wk_sb = wqkv_p.tile([128, KT, 128], BF16, name="wk_sb")
        nc.scalar.dma_start(out=wk_sb, in_=wk[:, :].rearrange("(k p) m -> p k m", p=128))
        wv_sb = wqkv_p.tile([128, KT, 128], BF16, name="wv_sb")
        nc.scalar.dma_start(out=wv_sb, in_=wv[:, :].rearrange("(k p) m -> p k m", p=128))
        hTe_p = tc.alloc_tile_pool(name="hTe", bufs=2)


        # =======================================================
        # P2: LN1 on local rows, PE-transpose to h^T, AllGather
        # =======================================================
        with tc.tile_pool(name="p2", bufs=4) as p2, \
             tc.tile_pool(name="p2big", bufs=1) as p2big:
            hT_sb = p2big.tile([128, KT, ROWS], BF16)
            for m in range(4):
                xt = xr_p.tile([128, C], F32, tag="xr", name="xr")
                nc.sync.dma_start(out=xt, in_=x_loc[128 * m:128 * (m + 1), :])
                h_t = p2.tile([128, C], BF16, tag="h", name="h_t")
                _ln_apply(nc, p2, xt, h_t, eps_tile)
                for kp in range(2):  # two groups of 4 k-tiles
                    pst = psT.tile([128, 4, 128], BF16, tag="tps", name="pst")
                    for kk in range(4):
                        k = 4 * kp + kk
                        nc.tensor.transpose(
                            pst[:, kk, :], h_t[:, 128 * k:128 * (k + 1)], identity)
                    nc.vector.tensor_copy(
                        out=hT_sb[:, 4 * kp:4 * (kp + 1), 128 * m:128 * (m + 1)],
                        in_=pst)
            for k in range(KT):
                nc.sync.dma_start(
                    out=hT_bounce[128 * k:128 * (k + 1), :], in_=hT_sb[:, k, :])
        nc.gpsimd.collective_compute(
            "AllGather", ALU.bypass, replica_groups=RG,
            ins=[hT_bounce[:, :]], outs=[hT_all[:, :, :]],
        )

        # w1/w2 MLP weight streaming pools (allocated late, in pipeline)
        mlp_pools = {}

        def qkv_batch(b):
            """QKV projections for batch b (4 chunks of 512 tokens)."""
            ap = attn_pools[b]
            qT = ap.tile([128, T], BF16, tag="qT", name=f"qTb{b}")
            kT = ap.tile([128, T], BF16, tag="kT", name=f"kTb{b}")
            vv = ap.tile([128, NBLK, 130], BF16, tag="vv", name=f"vvb{b}")
            nc.gpsimd.memset(vv[:, :, 64:65], 1.0)
            nc.gpsimd.memset(vv[:, :, 129:130], 1.0)
            for le in range(NCH):
                hTe = hTe_p.tile([128, KT, TQ], BF16, tag="hTe", name="hTe")
                # batch-b tokens [512*le, 512*(le+1)) live as rows
                # [256*b, 256*(b+1)) of cores 2*le and 2*le+1
                for hh in range(2):
                    nc.gpsimd.dma_start(
                        out=hTe[:, :, RPB * hh:RPB * (hh + 1)],
                        in_=hT_all[2 * le + hh][:, RPB * b:RPB * (b + 1)]
                        .rearrange("(k p) t -> p k t", p=128))
                for which, w_sb, bias_sb, dst in (
                    ("q", wq_sb, qb_sb, qT), ("k", wk_sb, kb_sb, kT),
                ):
                    ps = mmps.tile([128, TQ], F32, tag="mm", name=f"ps{which}")
                    for k in range(KT):
                        nc.tensor.matmul(ps, lhsT=w_sb[:, k, :], rhs=hTe[:, k, :],
                                         start=(k == 0), stop=(k == KT - 1))
                    nc.scalar.activation(
                        out=dst[:, TQ * le:TQ * (le + 1)], in_=ps,
                        func=AF.Identity, bias=bias_sb, scale=1.0)
                psv = mmps.tile([128, TQ], F32, tag="mm", name="psv")
                for k in range(KT):
                    nc.tensor.matmul(psv, lhsT=wv_sb[:, k, :], rhs=hTe[:, k, :],
                                     start=(k == 0), stop=(k == KT - 1))
                vTc = hTe_p.tile([128, TQ], BF16, tag="vTc", name="vTc")
                nc.scalar.copy(out=vTc, in_=psv)
                pst = psT.tile([128, 4, 128], BF16, tag="tps", name="pstv")
                for sb in range(4):
                    nc.tensor.transpose(
                        pst[:, sb, :], vTc[:, 128 * sb:128 * (sb + 1)], identity)
                tb0 = le * 4
                nc.vector.tensor_copy(
                    out=vv[:, tb0:tb0 + 4, 0:64], in_=pst[:, :, 0:64])
                nc.vector.tensor_copy(
                    out=vv[:, tb0:tb0 + 4, 65:129], in_=pst[:, :, 64:128])
            return qT, kT, vv

        def attn_batch(b, qT, kT, vv):
            """Causal attention for batch b; writes ynorm[b] (unnormalized
            for now) and sums_pack[b]."""
            for j in range(NCH):
                tq0 = TQ * j
                psy = [psY.tile([128, TQ], F32, tag=f"y{h}", name=f"psy{h}")
                       for h in range(HPC)]
                nblocks = 4 * j + 4
                for i in range(nblocks):
                    qoff = max(0, TB * (i - 4 * j))
                    for h in range(HPC):
                        pss = mmps.tile([128, TQ], F32, tag="mm", name=f"pss{h}")
                        nc.tensor.matmul(
                            pss[:, qoff:TQ],
                            lhsT=kT[64 * h:64 * (h + 1), TB * i:TB * (i + 1)],
                            rhs=qT[64 * h:64 * (h + 1), tq0 + qoff:tq0 + TQ],
                            start=True, stop=True)
                        pt = p5p.tile([128, TQ], BF16, tag=f"p{h}", name=f"pt{h}", bufs=3)
                        nc.scalar.activation(
                            out=pt[:, qoff:TQ], in_=pss[:, qoff:TQ],
                            func=AF.Exp, bias=0.0, scale=SCALE)
                        if i >= 4 * j:
                            nc.vector.tensor_mul(
                                out=pt[:, qoff:qoff + TB],
                                in0=pt[:, qoff:qoff + TB], in1=mask_sb)
                        nc.tensor.matmul(
                            psy[h][0:65, qoff:TQ],
                            lhsT=vv[:, i, 65 * h:65 * (h + 1)],
                            rhs=pt[:, qoff:TQ],
                            start=(i == 0), stop=(i == nblocks - 1))
                for h in range(HPC):
                    cb = h * NCH + j  # 0..7 within batch
                    srow = small.tile([1, TQ], F32, tag="srow", name="srow")
                    nc.vector.tensor_copy(out=srow, in_=psy[h][64:65, 0:TQ])
                    nc.sync.dma_start(
                        out=sums_pack[b][8 * cb:8 * (cb + 1), 0:64], in_=srow)
                    nc.vector.tensor_copy(
                        out=ynorm[b][64 * h:64 * (h + 1), tq0:tq0 + TQ],
                        in_=psy[h][0:64, :])

        def normalize_a2a(b):
            """Reciprocal + normalize ynorm[b] in place, bounce + AllToAll."""
            recip = small.tile([128, 64], F32, tag="recip", name="recip")
            nc.vector.reciprocal(recip[0:64, :], sums_pack[b][0:64, :])
            recip_bf = small.tile([128, 64], BF16, tag="recipb", name="recipb")
            nc.vector.tensor_copy(recip_bf[0:64, :], recip[0:64, :])
            nc.sync.dma_start(out=recip_dram[b][:, :], in_=recip_bf[0:64, :])
            for j in range(NCH):
                tq0 = TQ * j
                rep = p5p.tile([128, TQ], BF16, tag="rep", name="rep")
                for h in range(HPC):
                    cb = h * NCH + j
                    bc = bass.AP(tensor=recip_dram[b], offset=TQ * cb,
                                 ap=[[0, 64], [1, TQ]])
                    nc.sync.dma_start(out=rep[64 * h:64 * (h + 1), :], in_=bc)
                nc.vector.tensor_mul(
                    out=ynorm[b][:, tq0:tq0 + TQ],
                    in0=ynorm[b][:, tq0:tq0 + TQ], in1=rep)
            for e in range(N_CORES):
                nc.sync.dma_start(
                    out=y_bounce[b][e], in_=ynorm[b][:, RPB * e:RPB * (e + 1)])
            nc.gpsimd.collective_compute(
                "AllToAll", ALU.bypass, replica_groups=RG,
                ins=[y_bounce[b][:, :, :]], outs=[y_all[b][:, :, :]],
            )
            nc.gpsimd.dma_start(
                out=yT_recv[b], in_=y_all[b][:, :, :].rearrange("e p t -> p e t"))

        def mlp_front(b):
            """out-proj + residual + LN2 + mm1 + gelu for batch b rows."""
            for m in range(2):
                xm = 2 * b + m
                xt = xr_p.tile([128, C], F32, tag="xr", name="xr2")
                nc.sync.dma_start(
                    out=xt, in_=x_loc[128 * xm:128 * (xm + 1), :])
                for n in range(2):
                    ps = mmps.tile([128, TQ], F32, tag="mm", name="pso")
                    for e in range(N_CORES):
                        nc.tensor.matmul(
                            ps, lhsT=yT_recv[b][:, e, 128 * m:128 * (m + 1)],
                            rhs=wo_sb[:, e, TQ * n:TQ * (n + 1)],
                            start=(e == 0), stop=(e == N_CORES - 1))
                    nc.vector.tensor_add(
                        out=x2_sb[xm][:, TQ * n:TQ * (n + 1)],
                        in0=xt[:, TQ * n:TQ * (n + 1)], in1=ps)
                if add_bo:
                    nc.vector.tensor_add(
                        out=x2_sb[xm], in0=x2_sb[xm], in1=bo_rep)
                h2 = p5p.tile([128, C], BF16, tag="h2", name="h2")
                _ln_apply(nc, small, x2_sb[xm], h2, eps_tile)
                for kp in range(2):
                    pst = psT.tile([128, 4, 128], BF16, tag="tps", name="psth")
                    for kk in range(4):
                        k = 4 * kp + kk
                        nc.tensor.transpose(
                            pst[:, kk, :], h2[:, 128 * k:128 * (k + 1)], identity)
                    nc.vector.tensor_copy(
                        out=h2T_sb[:, 4 * kp:4 * (kp + 1), 128 * xm:128 * (xm + 1)],
                        in_=pst)
            # mm1 + gelu over this batch's 256 rows, W1 streamed in quarters
            w1p, gTp, mlp_p = (mlp_pools[k] for k in ("w1p", "gTp", "mlp"))
            gT = gTp.tile([128, 32, RPB], BF16, tag="gT", name=f"gT{b}")
            r0 = RPB * b
            for uq in range(8):
                w1q = w1p.tile([128, KT, 512], BF16, tag="w1q", name="w1q")
                nc.scalar.dma_start(
                    out=w1q,
                    in_=w1[:, 512 * uq:512 * (uq + 1)]
                    .rearrange("(k p) n -> p k n", p=128))
                for u8 in range(4):
                    u = 4 * uq + u8
                    ps = mmps.tile([128, RPB], F32, tag="mm", name="psu")
                    for k in range(KT):
                        nc.tensor.matmul(
                            ps, lhsT=w1q[:, k, 128 * u8:128 * (u8 + 1)],
                            rhs=h2T_sb[:, k, r0:r0 + RPB],
                            start=(k == 0), stop=(k == KT - 1))
                    nc.scalar.activation(
                        out=gT[:, u, :], in_=ps, func=AF.Gelu_apprx_tanh,
                        bias=gelub_sb[:, u:u + 1], scale=1.0)
            return gT

        def mlp_mm2(b, gT):
            # mm2: half-outer with streamed W2 half, partial accum in out tile
            mlp_p = mlp_pools["mlp"]
            ot_t = [mlp_p.tile([128, C], F32, tag=f"ot{m}", name=f"ot{m}")
                    for m in range(2)]
            for quarter in range(4):
                w2h = w2p.tile([128, 8, C], BF16, tag="w2h", name="w2h")
                nc.scalar.dma_start(
                    out=w2h, in_=w2[1024 * quarter:1024 * (quarter + 1), :]
                    .rearrange("(k p) n -> p k n", p=128))
                for m in range(2):
                    xm = 2 * b + m
                    for n in range(2):
                        ps = mmps.tile([128, TQ], F32, tag="mm", name="psm")
                        for kk in range(8):
                            u = 8 * quarter + kk
                            nc.tensor.matmul(
                                ps, lhsT=gT[:, u, 128 * m:128 * (m + 1)],
                                rhs=w2h[:, kk, TQ * n:TQ * (n + 1)],
                                start=(kk == 0), stop=(kk == 7))
                        if quarter == 0:
                            nc.vector.tensor_add(
                                out=ot_t[m][:, TQ * n:TQ * (n + 1)],
                                in0=x2_sb[xm][:, TQ * n:TQ * (n + 1)], in1=ps)
                        else:
                            nc.vector.tensor_add(
                                out=ot_t[m][:, TQ * n:TQ * (n + 1)],
                                in0=ot_t[m][:, TQ * n:TQ * (n + 1)], in1=ps)
            for m in range(2):
                xm = 2 * b + m
                if add_bb2:
                    nc.vector.tensor_add(out=ot_t[m], in0=ot_t[m], in1=bb2_rep)
                nc.sync.dma_start(
                    out=out_loc[128 * xm:128 * (xm + 1), :], in_=ot_t[m])

        # ---------------- pipeline ----------------
        qT0, kT0, vv0 = qkv_batch(0)
        attn_batch(0, qT0, kT0, vv0)
        qT1, kT1, vv1 = qkv_batch(1)       # fills PE idle during attn(0)
        hTe_p.release()
        wqkv_p.release()
        mlp_pools["w1p"] = tc.alloc_tile_pool(name="w1p", bufs=2)
        mlp_pools["gTp"] = tc.alloc_tile_pool(name="gTp", bufs=1)
        mlp_pools["mlp"] = tc.alloc_tile_pool(name="mlp", bufs=2)
        w2p = tc.alloc_tile_pool(name="w2p", bufs=2)
        normalize_a2a(0)
        attn_batch(1, qT1, kT1, vv1)
        gT0 = mlp_front(0)                  # overlaps attn(1)
        normalize_a2a(1)
        mlp_mm2(0, gT0)                     # covers A2A(1) transit
        gT1 = mlp_front(1)
        mlp_mm2(1, gT1)
        for pool in (w2p, mlp_pools["mlp"], mlp_pools["gTp"],
                     mlp_pools["w1p"], xr_p, attn_pools[1], attn_pools[0],
                     p5p, psY, yiop, wop, xp):
            pool.release()

    nc.finalize()
    return nc


_program_cache = {}


def kernel(**inputs) -> np.ndarray:
    global last_results
    x = np.asarray(inputs["x"], np.float32)
    Wq = np.asarray(inputs["Wq"], np.float32)
    Wk = np.asarray(inputs["Wk"], np.float32)
    Wv = np.asarray(inputs["Wv"], np.float32)
    Wo = np.asarray(inputs["Wo"], np.float32)
    bo = np.asarray(inputs["bo"], np.float32)
    g1 = np.asarray(inputs["g1"], np.float32)
    b1 = np.asarray(inputs["b1"], np.float32)
    g2 = np.asarray(inputs["g2"], np.float32)
    b2 = np.asarray(inputs["b2"], np.float32)
    W1 = np.asarray(inputs["W1"], np.float32)
    bb1 = np.asarray(inputs["bb1"], np.float32)
    W2 = np.asarray(inputs["W2"], np.float32)
    bb2 = np.asarray(inputs["bb2"], np.float32)

    xb = x.reshape(B, T, C)

    vb_full = np.einsum("c,hcd->hd", b1, Wv).reshape(C)
    boeff = bo + vb_full @ Wo
    gelu_bias = b2 @ (g2[:, None] * W1) + bb1
    add_bo = bool(np.abs(boeff).max() > 0)
    add_bb2 = bool(np.abs(bb2).max() > 0)

    key = (add_bo, add_bb2)
    if key not in _program_cache:
        _program_cache[key] = build_program(add_bo, add_bb2)
    nc = _program_cache[key]

    wo_b = Wo.astype(bf16)
    w1_b = (g2[:, None] * W1).astype(bf16)
    w2_b = W2.astype(bf16)
    gelub_np = np.ascontiguousarray(
        gelu_bias.reshape(32, 128).T).astype(np.float32)
    s_idx = np.arange(128)[:, None]
    q_idx = np.arange(128)[None, :]
    mask_np = (s_idx <= q_idx).astype(bf16)

    in_maps = []
    for c in range(N_CORES):
        hsl = slice(HPC * c, HPC * (c + 1))
        Wqh = (g1[:, None] * Wq[hsl]).transpose(1, 0, 2).reshape(C, 128)
        Wkh = (g1[:, None] * Wk[hsl]).transpose(1, 0, 2).reshape(C, 128)
        Wvh = (g1[:, None] * Wv[hsl]).transpose(1, 0, 2).reshape(C, 128)
        qb_np = np.einsum("c,hcd->hd", b1, Wq[hsl]).reshape(128, 1)
        kb_np = np.einsum("c,hcd->hd", b1, Wk[hsl]).reshape(128, 1)
        x_core = np.concatenate(
            [xb[b, RPB * c:RPB * (c + 1), :] for b in range(B)], axis=0)
        m = {
            "x_loc": np.ascontiguousarray(x_core),
            "wq": np.ascontiguousarray(Wqh).astype(bf16),
            "wk": np.ascontiguousarray(Wkh).astype(bf16),
            "wv": np.ascontiguousarray(Wvh).astype(bf16),
            "qb": qb_np.astype(np.float32), "kb": kb_np.astype(np.float32),
            "wo": wo_b, "w1": w1_b, "w2": w2_b,
            "gelub": gelub_np, "maskin": mask_np,
        }
        if add_bo:
            m["boeff"] = boeff.reshape(1, C).astype(np.float32)
        if add_bb2:
            m["bb2v"] = bb2.reshape(1, C).astype(np.float32)
        in_maps.append(m)

    trace = bool(int(os.environ.get("KERNEL_TRACE", "0")))
    res = run_bass_kernel_spmd(nc, in_maps, core_ids=list(range(N_CORES)),
                               trace=trace)
    last_results = res
    out = np.empty((B, T, C), np.float32)
    for c in range(N_CORES):
        r = res.results[c]["out_loc"]
        for b in range(B):
            out[b, RPB * c:RPB * (c + 1), :] = r[RPB * b:RPB * (b + 1), :]
    return out
